# revision 11
# baseline (speedup 1.0000x reference)
"""KNN attention kernel for 8 NeuronCores (Trainium2, Bass/Tile).

Strategy (data-parallel over queries):
  - Each core owns N1/8 query points. coords0/feats0 (the gather source) are
    packed on the host into one fp16 table `tbl [N0, 128]` whose partition
    layout is: rows 0..15 = tag zone (zeros), 16..79 = feats0, 80..82 =
    coords0, 83..127 = zero pad.  A row is 256 bytes, the granularity of
    `dma_gather`.
  - `dma_gather(transpose=True)` delivers gathered rows feature-major:
    G[128 partitions, cols] fp16, one column per (query, neighbor) pair.
    dma_gather indices are int16, so each core's pairs are grouped into 4
    bucket segments (idx // 32768) with bucket-local indices.  Within a
    bucket segment, columns are laid out per (16-query half-window) cell with
    a fixed capacity = max count over the 8 cores (shared SPMD program),
    padded with idx 0 (masked out later).
  - Per 128-col chunk, one PE matmul against rhs [83, 129] computes both
    scores (cols 0..63 = 16 queries x 4 heads, via a host-precomputed
    q-tilde = Wq Wk^T folding that skips the K projection) and the V
    projection + a denominator column.  Tag rows (partitions 0..15 of G,
    written by a host-supplied one-hot DMA) add +30 to each column's own
    query's scores; exp(bias=-30) then zeroes every cross-query term.
  - exp -> E (fp16), then a second matmul E^T @ [V|den] accumulates the
    attention numerator/denominator per 32-query window in PSUM.
  - Host divides, adds the per-query affine term (bv - c1 @ Wv_coords, and
    the softmax-invariant query-side constants drop), and unshards.
"""

import math
import numpy as np

import concourse.bacc as bacc
import concourse.tile as tile
from concourse import bass, mybir
from concourse.bass_utils import run_bass_kernel_spmd

# problem constants
N0 = 100000
N1 = 100000
K = 16
DM = 64
DA = 32
H = 4
NCORES = 8

# grid constants
BUCKET = 32768
NBUCK = 4
BLKQ = 512
WINQ = 32
HALFQ = 16
COFF = 0            # first data row (partition) in the table / G
CDIM = 67           # kv feature count (64 feats + 3 coords)
TOFF = 96           # tag zone partition offset (16 rows; offset!=0 APs max 32 parts)
KDIM = TOFF + 16    # matmul contraction rows = 112
TAG = 30.0
F16 = mybir.dt.float16
F32 = mybir.dt.float32
I16 = mybir.dt.int16


class _Grid:
    pass


def _ceil_to(x, m):
    return (x + m - 1) // m * m


def _build_grid(idx_all, nloc, n0):
    """idx_all: [NCORES, nloc, K] int. Returns shared grid + per-core layouts."""
    g = _Grid()
    g.nloc = nloc
    g.nbuck = max(1, -(-n0 // BUCKET))
    nblk = -(-nloc // BLKQ)
    g.nblk = nblk
    g.blk_q0 = [b * BLKQ for b in range(nblk)]
    g.blk_nq = [min(BLKQ, nloc - b * BLKQ) for b in range(nblk)]
    g.blk_nwin = [-(-nq // WINQ) for nq in g.blk_nq]

    nb = g.nbuck
    # cell existence and counts: [nblk, nb, 16, 2]
    counts = np.zeros((NCORES, nblk, nb, WINQ // HALFQ * 32 // 32 * 16, 2), np.int64)
    counts = np.zeros((NCORES, nblk, nb, 16, 2), np.int64)
    percore = []
    for c in range(NCORES):
        I = np.asarray(idx_all[c]).astype(np.int64)
        q = np.repeat(np.arange(nloc), K)
        iv = I.reshape(-1)
        bck = iv // BUCKET
        blk = q // BLKQ
        qin = q % BLKQ
        win = qin // WINQ
        half = (qin % WINQ) // HALFQ
        qslot = qin % HALFQ
        key = (((blk * nb + bck) * 16 + win) * 2 + half)
        cnt = np.bincount(key, minlength=nblk * nb * 16 * 2).reshape(nblk, nb, 16, 2)
        counts[c] = cnt
        percore.append((iv, q, bck, blk, win, half, qslot, key))

    caps = counts.max(axis=0)
    # existence mask
    for b in range(nblk):
        nq = g.blk_nq[b]
        for w in range(16):
            for hf in range(2):
                base = w * WINQ + hf * HALFQ
                if w >= g.blk_nwin[b] or base >= nq:
                    caps[b, :, w, hf] = -1  # nonexistent
    exists = caps >= 0
    caps = np.where(exists, np.maximum(caps, 2), 0)
    # pad each (blk, bucket) segment total to a multiple of 128
    for b in range(nblk):
        for bk in range(nb):
            tot = caps[b, bk].sum()
            r = (-tot) % 128
            if r:
                wz, hz = np.argwhere(exists[b, bk])[-1]
                caps[b, bk, wz, hz] += r
    g.caps = caps

    # column offsets (block-local), cell order = (bucket, win, half)
    cell_off = np.zeros((nblk, nb, 16, 2), np.int64)
    g.call_off = np.zeros((nblk, nb), np.int64)
    g.call_len = np.zeros((nblk, nb), np.int64)
    g.blk_totc = np.zeros(nblk, np.int64)
    for b in range(nblk):
        off = 0
        for bk in range(nb):
            g.call_off[b, bk] = off
            for w in range(16):
                for hf in range(2):
                    cell_off[b, bk, w, hf] = off
                    off += caps[b, bk, w, hf]
            g.call_len[b, bk] = off - g.call_off[b, bk]
        g.blk_totc[b] = off
    g.cell_off = cell_off
    g.blk_off = np.concatenate([[0], np.cumsum(g.blk_totc)])
    g.qtot = int(g.blk_off[-1])
    g.totc_max = int(g.blk_totc.max())

    # uout column offsets
    g.ublk_off = np.concatenate([[0], np.cumsum(np.array(g.blk_nwin) * 65)])
    g.uw = int(g.ublk_off[-1])

    # per-core idx / tag arrays
    g.idxw = []
    g.tags = []
    for c in range(NCORES):
        iv, q, bck, blk, win, half, qslot, key = percore[c]
        order = np.lexsort((np.tile(np.arange(K), nloc), q, key))
        sk = key[order]
        run_start = np.zeros(len(sk), np.int64)
        new = np.ones(len(sk), bool)
        new[1:] = sk[1:] != sk[:-1]
        starts = np.flatnonzero(new)
        run_id = np.cumsum(new) - 1
        rank = np.arange(len(sk)) - starts[run_id]
        colpos = np.empty(len(sk), np.int64)
        colpos[order] = cell_off[blk[order], bck[order], win[order], half[order]] + rank
        gpos = g.blk_off[blk] + colpos
        idxw = np.zeros((16, g.qtot // 16), np.int16)
        idxw[gpos % 16, gpos // 16] = (iv - bck * BUCKET).astype(np.int16)
        idxw = np.tile(idxw, (8, 1))
        tags = np.zeros((16, g.qtot), np.float16)
        tags[qslot, gpos] = 1.0
        g.idxw.append(idxw)
        g.tags.append(tags)

    # chunk list per block: (col_off_local, ncols, win, half, first_in_half, last_in_half)
    g.blk_chunks = []
    for b in range(nblk):
        chunks = []
        for w in range(g.blk_nwin[b]):
            for hf in range(2):
                cell_chunks = []
                for bk in range(nb):
                    cap = int(caps[b, bk, w, hf])
                    off = int(cell_off[b, bk, w, hf])
                    while cap > 0:
                        n = min(cap, 128)
                        cell_chunks.append((off, n))
                        off += n
                        cap -= n
                for i, (off, n) in enumerate(cell_chunks):
                    chunks.append((off, n, w, hf, i == 0, i == len(cell_chunks) - 1))
        g.blk_chunks.append(chunks)
    return g


def _host_consts(Wq, bq, Wk, bk, Wv, bv):
    cst = np.zeros((128, 129), np.float32)
    for qh in range(HALFQ):
        for hd in range(H):
            cst[TOFF + qh, 4 * qh + hd] = TAG
    cst[TOFF:TOFF + 16, 128] = 1.0
    cst[COFF:COFF + CDIM, 64:64 + DM] = Wv
    mmt = np.zeros((DM + 1, 4 * 128), np.float32)
    s = 1.0 / math.sqrt(DA // H)
    for h in range(H):
        Wqh = Wq[:, 8 * h:8 * h + 8]
        Wkh = Wk[:, 8 * h:8 * h + 8]
        mmt[0:DM, 128 * h + COFF:128 * h + COFF + CDIM] = s * (Wqh @ Wkh.T)
        mmt[DM, 128 * h + COFF:128 * h + COFF + CDIM] = s * (Wkh @ bq[8 * h:8 * h + 8])
    return cst.astype(np.float16), mmt.astype(np.float16)


def _host_prep(coords0, coords1, feats0, feats1, knn, nloc=None, n0=None):
    nloc = nloc or N1 // NCORES
    n0 = n0 or N0
    idx = np.asarray(knn)[0]
    idx_all = [idx[c * nloc:(c + 1) * nloc] for c in range(NCORES)]
    g = _build_grid(idx_all, nloc, n0)

    tbl = np.zeros((n0, 128), np.float16)
    tbl[:, COFF:COFF + DM] = feats0
    tbl[:, COFF + DM:COFF + DM + 3] = coords0
    g.tbl = tbl

    g.f1t = []
    for c in range(NCORES):
        f = np.ones((DM + 1, nloc), np.float16)
        f[0:DM] = np.asarray(feats1[c * nloc:(c + 1) * nloc]).T
        g.f1t.append(f)
    return g


def _emulate_core(g, c, cst, mmt):
    """Numpy emulation of the device program for core c -> uout [128, uw]."""
    uout = np.zeros((128, g.uw), np.float32)
    cstf = cst.astype(np.float32)
    for b in range(g.nblk):
        totc = int(g.blk_totc[b])
        boff = int(g.blk_off[b])
        G = np.zeros((128, totc), np.float16)
        for bk in range(g.nbuck):
            off, L = int(g.call_off[b, bk]), int(g.call_len[b, bk])
            ii = np.arange(L)
            gp = boff + off + ii
            li = g.idxw[c][gp % 16, gp // 16].astype(np.int64)
            G[:, off:off + L] = g.tbl[bk * BUCKET + li].T
        G[TOFF:TOFF + 16, :] = g.tags[c][:, boff:boff + totc]
        nq = g.blk_nq[b]
        q0 = g.blk_q0[b]
        f1 = g.f1t[c][:, q0:q0 + nq].astype(np.float32)
        qsb = np.zeros((128, 4 * nq), np.float16)
        for h in range(H):
            qp = mmt[:, 128 * h:128 * h + 128].astype(np.float32).T @ f1
            qsb[:, h::4] = qp.astype(np.float16)
        for w in range(g.blk_nwin[b]):
            up = np.zeros((128, 65), np.float32)
            for (off, n, w2, hf, first, last) in g.blk_chunks[b]:
                if w2 != w:
                    continue
                rhs = cstf.copy()
                base = (WINQ * w + HALFQ * hf) * 4
                wcap = min(64, 4 * nq - base)
                sl = qsb[COFF:COFF + CDIM, base:base + wcap].astype(np.float32)
                rhs[COFF:COFF + CDIM, 0:wcap] = sl
                sp = G[0:KDIM, off:off + n].astype(np.float32).T @ rhs[0:KDIM]
                em = np.exp(sp[:, 0:64] - TAG).astype(np.float16)
                vt = sp[:, 64:129].astype(np.float16)
                up[64 * hf:64 * hf + 64] += em.astype(np.float32).T @ vt.astype(np.float32)
            uout[:, int(g.ublk_off[b]) + 65 * w: int(g.ublk_off[b]) + 65 * (w + 1)] = up
    return uout


def _postprocess(g, uouts, coords1, Wv, bv, nloc):
    w1 = bv[None, :] - np.asarray(coords1) @ Wv[DM:DM + 3]
    out = np.zeros((NCORES * nloc, DM), np.float32)
    for c in range(NCORES):
        for b in range(g.nblk):
            nw = g.blk_nwin[b]
            U = uouts[c][:, int(g.ublk_off[b]):int(g.ublk_off[b]) + 65 * nw]
            U = U.reshape(2, HALFQ, H, nw, 65)  # [half, qslot, hd, win, 65]
            den = U[..., 64]
            for hd in range(H):
                num = U[:, :, hd, :, 16 * hd:16 * hd + 16]
                dn = den[:, :, hd, :]
                hf, qs, wn = np.meshgrid(np.arange(2), np.arange(HALFQ),
                                         np.arange(nw), indexing="ij")
                n = g.blk_q0[b] + WINQ * wn + HALFQ * hf + qs
                ok = n < nloc
                gi = c * nloc + n[ok]
                out[gi, 16 * hd:16 * hd + 16] = num[ok] / dn[ok][:, None]
    return out + w1


def _build_program(g, n0):
    nc = bacc.Bacc("TRN2", target_bir_lowering=False, debug=False,
                   num_devices=NCORES)
    t_tbl = nc.dram_tensor("tbl", [n0, 128], F16, kind="ExternalInput").ap()
    t_f1t = nc.dram_tensor("f1t", [DM + 1, g.nloc], F16, kind="ExternalInput").ap()
    t_idx = nc.dram_tensor("idx", [128, g.qtot // 16], I16, kind="ExternalInput").ap()
    t_tgs = nc.dram_tensor("tgs", [16, g.qtot], F16, kind="ExternalInput").ap()
    t_cst = nc.dram_tensor("cst", [128, 129], F16, kind="ExternalInput").ap()
    t_mmt = nc.dram_tensor("mmt", [DM + 1, 512], F16, kind="ExternalInput").ap()
    t_uout = nc.dram_tensor("uout", [128, g.uw], F32, kind="ExternalOutput").ap()

    with tile.TileContext(nc) as tc:
        import contextlib
        with contextlib.ExitStack() as ctx:
            cpool = ctx.enter_context(tc.tile_pool(name="const", bufs=1))
            gpool = ctx.enter_context(tc.tile_pool(name="g", bufs=2))
            ipool = ctx.enter_context(tc.tile_pool(name="idx", bufs=2))
            fpool = ctx.enter_context(tc.tile_pool(name="f1", bufs=2))
            qspool = ctx.enter_context(tc.tile_pool(name="qsb", bufs=2))
            rpool = ctx.enter_context(tc.tile_pool(name="rhs", bufs=4))
            empool = ctx.enter_context(tc.tile_pool(name="em", bufs=4))
            vtpool = ctx.enter_context(tc.tile_pool(name="vt", bufs=4))
            uspool = ctx.enter_context(tc.tile_pool(name="ustg", bufs=2))
            qppool = ctx.enter_context(tc.tile_pool(name="qpsum", bufs=2, space="PSUM"))
            sppool = ctx.enter_context(tc.tile_pool(name="spsum", bufs=2, space="PSUM"))
            uppool = ctx.enter_context(tc.tile_pool(name="upsum", bufs=2, space="PSUM"))

            cst_t = cpool.tile([128, 129], F16)
            nc.sync.dma_start(cst_t[:], t_cst[:])
            mmt_t = cpool.tile([DM + 1, 512], F16)
            nc.sync.dma_start(mmt_t[:], t_mmt[:])
            bias_t = cpool.tile([128, 1], F32)
            nc.vector.memset(bias_t[:], -TAG)

            for b in range(g.nblk):
                totc = int(g.blk_totc[b])
                boff = int(g.blk_off[b])
                nq = g.blk_nq[b]
                q0 = g.blk_q0[b]

                G = gpool.tile([128, g.totc_max], F16, tag="g")
                it = ipool.tile([128, g.totc_max // 16], I16, tag="idx")
                nc.sync.dma_start(it[:, :totc // 16],
                                  t_idx[:, boff // 16:(boff + totc) // 16])
                for bk in range(g.nbuck):
                    off, L = int(g.call_off[b, bk]), int(g.call_len[b, bk])
                    r0 = bk * BUCKET
                    r1 = min(n0, r0 + BUCKET)
                    src = t_tbl[r0:r1, :]
                    gout = bass.AP(G.tensor, G.offset + off,
                                   [G.ap[0], (L, 1), (1, L)])
                    nc.gpsimd.dma_gather(
                        out_ap=gout, in_ap=src,
                        idxs_ap=it[:, off // 16:(off + L) // 16],
                        num_idxs=L, num_idxs_reg=L,
                        elem_size=128, elem_step=128, transpose=True,
                        single_packet=False)
                    nc.sync.dma_start(G[TOFF:TOFF + 16, off:off + L],
                                      t_tgs[:, boff + off:boff + off + L])

                f1 = fpool.tile([DM + 1, BLKQ], F16, tag="f1")
                nc.sync.dma_start(f1[:, :nq], t_f1t[:, q0:q0 + nq])
                qsb = qspool.tile([128, 4 * BLKQ], F16, tag="qsb")
                for h in range(H):
                    qp = qppool.tile([128, BLKQ], F32, tag="qpsum")
                    nc.tensor.matmul(out=qp[:, :nq],
                                     lhsT=mmt_t[:, 128 * h:128 * (h + 1)],
                                     rhs=f1[:, :nq], start=True, stop=True)
                    qout = bass.AP(qsb.tensor, qsb.offset + h,
                                   [(qsb.ap[0][0], CDIM), (4, nq)])
                    nc.vector.tensor_copy(out=qout, in_=qp[0:CDIM, :nq])

                ustg = [uspool.tile([64, 16 * 65], F32, tag=f"ustg{i}",
                                    name=f"ustg{i}_{b}")
                        for i in range(2)]
                nchunk = 0
                for w in range(g.blk_nwin[b]):
                    up = [uppool.tile([64, 65], F32, tag=f"upsum{i}",
                                      name=f"up{i}_{b}_{w}")
                          for i in range(2)]
                    cur_rhs = {}
                    for (off, n, w2, hf, first, last) in g.blk_chunks[b]:
                        if w2 != w:
                            continue
                        if hf not in cur_rhs:
                            rb = rpool.tile([128, 129], F16, tag="rhs")
                            nc.vector.tensor_copy(out=rb[:], in_=cst_t[:])
                            base = (WINQ * w + HALFQ * hf) * 4
                            wcap = min(64, 4 * nq - base)
                            nc.vector.tensor_copy(
                                out=rb[0:CDIM, 0:wcap],
                                in_=qsb[0:CDIM, base:base + wcap])
                            cur_rhs[hf] = rb
                        rb = cur_rhs[hf]
                        sp = sppool.tile([128, 129], F32, tag="spsum")
                        nc.tensor.matmul(out=sp[0:n, :],
                                         lhsT=G[0:KDIM, off:off + n],
                                         rhs=rb[0:KDIM, :], start=True, stop=True)
                        em = empool.tile([128, 64], F16, tag="em")
                        nc.scalar.activation(out=em[0:n, :], in_=sp[0:n, 0:64],
                                             func=mybir.ActivationFunctionType.Exp,
                                             bias=bias_t[0:n, 0:1])
                        vt = vtpool.tile([128, 65], F16, tag="vt")
                        if nchunk % 2 == 0:
                            nc.vector.tensor_copy(out=vt[0:n, :], in_=sp[0:n, 64:129])
                        else:
                            nc.scalar.activation(out=vt[0:n, :], in_=sp[0:n, 64:129],
                                                 func=mybir.ActivationFunctionType.Copy)
                        nc.tensor.matmul(out=up[hf][:],
                                         lhsT=em[0:n, :], rhs=vt[0:n, :],
                                         start=first, stop=last,
                                         skip_group_check=True)
                        nchunk += 1
                    for i in range(2):
                        nc.vector.tensor_copy(
                            out=ustg[i][:, 65 * w:65 * (w + 1)], in_=up[i][:])
                uoff = int(g.ublk_off[b])
                nw = g.blk_nwin[b]
                nc.sync.dma_start(t_uout[0:64, uoff:uoff + 65 * nw],
                                  ustg[0][:, :65 * nw])
                nc.sync.dma_start(t_uout[64:128, uoff:uoff + 65 * nw],
                                  ustg[1][:, :65 * nw])
    nc.compile()
    return nc


LAST_RUN_INFO = {}


def _run(g, n0, cst, mmt, trace=False):
    import time
    t0 = time.monotonic()
    nc = _build_program(g, n0)
    t1 = time.monotonic()
    in_maps = []
    for c in range(NCORES):
        in_maps.append({
            "tbl": g.tbl, "f1t": np.ascontiguousarray(g.f1t[c]),
            "idx": g.idxw[c], "tgs": g.tags[c],
            "cst": cst, "mmt": mmt,
        })
    t2 = time.monotonic()
    res = run_bass_kernel_spmd(nc, in_maps, list(range(NCORES)), trace=trace)
    outs = [res.results[c]["uout"] for c in range(NCORES)]
    t3 = time.monotonic()
    LAST_RUN_INFO.update(build_s=t1 - t0, run_s=t3 - t2,
                         exec_time_ns=res.exec_time_ns,
                         profile=res.profile_json)
    return outs, res


def kernel(coords0, coords1, feats0, feats1, knn_idxs, Wq, bq, Wk, bk, Wv, bv,
           _trace=None):
    coords0 = np.asarray(coords0, np.float32)
    coords1 = np.asarray(coords1, np.float32)
    feats0 = np.asarray(feats0, np.float32)
    feats1 = np.asarray(feats1, np.float32)
    Wq = np.asarray(Wq, np.float32)
    bq = np.asarray(bq, np.float32)
    Wk = np.asarray(Wk, np.float32)
    Wv = np.asarray(Wv, np.float32)
    bv = np.asarray(bv, np.float32)
    nloc = N1 // NCORES
    g = _host_prep(coords0, coords1, feats0, feats1, knn_idxs)
    cst, mmt = _host_consts(Wq, bq, Wk, bk, Wv, bv)
    import os
    if _trace is None:
        _trace = bool(int(os.environ.get("KERNEL_TRACE", "0")))
    uouts, _ = _run(g, N0, cst, mmt, trace=_trace)
    out = _postprocess(g, uouts, coords1, Wv, bv, nloc)
    return out, np.asarray(knn_idxs)
